# revision 1
# baseline (speedup 1.0000x reference)
"""Causal self-attention (B=2, S=2048, D=1024, H=16, hd=64) on 8 TRN2 cores.

Sharding: core c = (b, hg) with b = c // 4, hg = c % 4. Each core computes
attention for heads [hg*4, hg*4+4) of batch b plus its partial output
projection (rows of Wo.T for those heads); the host sums the 4 partials
per batch (the tensor-parallel all-reduce, done at gather time).

Single-core kernel (SPMD, per-core data):
  xT [1024, 2048] (f32r)     x[b].T
  wq_t/wk_t [1024, 256]      W[heads].T with head-dims permuted [evens|odds]
  wv_t [1024, 256]           Wv[heads].T (no permutation)
  wo_t [256, 1024]           Wo[:, heads].T
  cosE/sinE [128, 2048]      RoPE tables in QT-row layout (sinE sign-baked)
  pswap [128, 128]           32-row block swap permutation
Pipeline: QKV projections (PE, f32r) -> RoPE (PE swap matmul + 3 DVE ops)
-> flash attention per (head, Sq-tile): scores^T = k @ q^T (PE), exp (ACT,
scale=1/8, bf16 out), causal fill on diagonal tiles (GPSIMD affine_select),
PV with ones-augmented V giving softmax denominators for free, divide
(DVE reciprocal_approx + GPSIMD partition_broadcast + DVE mult) -> output
projection (PE) -> DMA partial [2048, 1024] to DRAM.
"""
import numpy as np

N_CORES = 8
B, S, D, H, HD = 2, 2048, 1024, 16, 64
HPC = H // 4            # heads per core = 4
HS = HPC * HD           # head-dim slice per core = 256
NKC = D // 128          # K chunks for projections = 8
NST = S // 128          # S subtiles of 128 = 16
NSQ = S // 512          # Sq tiles of 512 = 4

_cached = {}


def _build_nc(reps=1):
    import concourse.bacc as bacc
    import concourse.mybir as mybir
    from concourse.tile import TileContext

    F32R = mybir.dt.float32r
    F32 = mybir.dt.float32
    BF16 = mybir.dt.bfloat16
    Exp = mybir.ActivationFunctionType.Exp

    nc = bacc.Bacc()
    xT = nc.declare_dram_parameter("xT", [D, S], F32R, isOutput=False)
    wq_t = nc.declare_dram_parameter("wq_t", [D, HS], F32R, isOutput=False)
    wk_t = nc.declare_dram_parameter("wk_t", [D, HS], F32R, isOutput=False)
    wv_t = nc.declare_dram_parameter("wv_t", [D, HS], F32R, isOutput=False)
    wo_t = nc.declare_dram_parameter("wo_t", [HS, D], F32R, isOutput=False)
    cosE = nc.declare_dram_parameter("cosE", [128, S], F32, isOutput=False)
    sinE = nc.declare_dram_parameter("sinE", [128, S], F32, isOutput=False)
    pswap = nc.declare_dram_parameter("pswap", [128, 128], F32R, isOutput=False)
    tick = nc.declare_dram_parameter("tick", [1, 1], F32, isOutput=False)
    out_p = nc.declare_dram_parameter("out_p", [S, D], F32, isOutput=True)
    tock = nc.declare_dram_parameter("tock", [1, 1], F32, isOutput=True)

    with TileContext(nc) as tc:
        nc.sync.dma_start(out=tock[:], in_=tick[:])
        for _rep in range(reps):
          with tc.tile_pool(name="qkv", bufs=1) as qkpool:

            # qt/kt chunks [128, S] f32r: chunk m holds heads (2m, 2m+1).
            qt = {}
            for wname in ("q", "k"):
                for m in range(2):
                    qt[(wname, m)] = qkpool.tile([128, S], F32R, name=f"{wname}t{m}")
            v_sb = [qkpool.tile([128, HPC, HD + 1], BF16, name=f"v{st}")
                    for st in range(NST)]

            # ================= phase 1: QKV + RoPE =================
            with tc.tile_pool(name="const", bufs=1) as cpool, \
                 tc.tile_pool(name="work", bufs=3) as wpool, \
                 tc.tile_pool(name="ps_qk", bufs=4, space="PSUM") as ps_qk, \
                 tc.tile_pool(name="ps_sw", bufs=2, space="PSUM") as ps_sw, \
                 tc.tile_pool(name="ps_v", bufs=2, space="PSUM") as ps_v:

                cos_t = cpool.tile([128, S], F32, name="cos_t")
                sin_t = cpool.tile([128, S], F32, name="sin_t")
                psw_t = cpool.tile([128, 128], F32R, name="psw_t")
                nc.sync.dma_start(out=cos_t[:], in_=cosE[:])
                nc.sync.dma_start(out=sin_t[:], in_=sinE[:])
                nc.sync.dma_start(out=psw_t[:], in_=pswap[:])

                xt_tiles = []
                for k in range(NKC):
                    t = cpool.tile([128, S], F32R, name=f"xt{k}")
                    nc.sync.dma_start(out=t[:], in_=xT[k * 128:(k + 1) * 128, :])
                    xt_tiles.append(t)

                w_tiles = {}
                for wname, wdram in [("q", wq_t), ("k", wk_t), ("v", wv_t)]:
                    for k in range(NKC):
                        t = cpool.tile([128, HS], F32R, name=f"w{wname}{k}")
                        nc.sync.dma_start(out=t[:],
                                          in_=wdram[k * 128:(k + 1) * 128, :])
                        w_tiles[(wname, k)] = t

                for wname in ("q", "k"):
                    for m in range(2):
                        for s in range(NSQ):
                            acc = ps_qk.tile([128, 512], F32, tag="qkacc")
                            for k in range(NKC):
                                nc.tensor.matmul(
                                    acc[:],
                                    w_tiles[(wname, k)][:, m * 128:(m + 1) * 128],
                                    xt_tiles[k][:, s * 512:(s + 1) * 512],
                                    start=(k == 0), stop=(k == NKC - 1),
                                )
                            sl = slice(s * 512, (s + 1) * 512)
                            # m1 = acc * sinE' (f32r); swap via PE; m2 = acc*cos
                            m1 = wpool.tile([128, 512], F32R, tag="rope_m1")
                            nc.vector.tensor_mul(m1[:], acc[:], sin_t[:, sl])
                            sw = ps_sw.tile([128, 512], F32, tag="swap")
                            nc.tensor.matmul(sw[:], psw_t[:], m1[:],
                                             start=True, stop=True)
                            m2 = wpool.tile([128, 512], F32, tag="rope_m2")
                            nc.vector.tensor_mul(m2[:], acc[:], cos_t[:, sl])
                            nc.vector.tensor_add(qt[(wname, m)][:, sl],
                                                 m2[:], sw[:])

                # V projection -> v_sb [128, 4, 65] bf16 with ones column
                for st in range(NST):
                    vt = v_sb[st]
                    nc.vector.memset(vt[:, :, HD:HD + 1], 1.0)
                    acc = ps_v.tile([128, HS], F32, tag="vacc")
                    for k in range(NKC):
                        nc.tensor.matmul(
                            acc[:],
                            xt_tiles[k][:, st * 128:(st + 1) * 128],
                            w_tiles[("v", k)][:],
                            start=(k == 0), stop=(k == NKC - 1),
                        )
                    nc.vector.tensor_copy(
                        vt[:, :, 0:HD],
                        acc[:].rearrange("p (h d) -> p h d", h=HPC),
                    )

            # ================= phase 2: attention =================
            with tc.tile_pool(name="attn", bufs=1) as apool, \
                 tc.tile_pool(name="ptile", bufs=3) as ppool, \
                 tc.tile_pool(name="dpool", bufs=3) as dpool:

                ot = [apool.tile([128, S], F32R, name=f"ot{m}") for m in range(2)]
                wo_tiles = []
                for m in range(2):
                    t = apool.tile([128, D], F32R, name=f"wo{m}")
                    nc.sync.dma_start(out=t[:], in_=wo_t[m * 128:(m + 1) * 128, :])
                    wo_tiles.append(t)

                with tc.tile_pool(name="ps_sc", bufs=1, space="PSUM") as ps_sc, \
                     tc.tile_pool(name="ps_pv", bufs=1, space="PSUM") as ps_pv:
                    for hp in range(2):          # head pair (= qt/kt chunk)
                        for sqh in range(2):     # sq half: {0,1} or {2,3}
                            sqs = [2 * sqh, 2 * sqh + 1]
                            pv_ps = {}
                            for hh in range(2):
                                for sq in sqs:
                                    pv_ps[(hh, sq)] = ps_pv.tile(
                                        [HD + 1, 512], F32,
                                        name=f"pv{hh}{sq % 2}",
                                        tag=f"pv{hh}{sq % 2}")
                            jmax = sqs[-1] * 4 + 3
                            for j in range(jmax + 1):
                                valid = [sq for sq in sqs if sq >= j // 4]
                                w = len(valid) * 512
                                for hh in range(2):
                                    hsl = slice(hh * 64, (hh + 1) * 64)
                                    sc = ps_sc.tile([128, 1024], F32,
                                                    tag=f"sc{hh}")
                                    for si, sq in enumerate(valid):
                                        nc.tensor.matmul(
                                            sc[:, si * 512:(si + 1) * 512],
                                            qt[("k", hp)][hsl,
                                                          j * 128:(j + 1) * 128],
                                            qt[("q", hp)][hsl,
                                                          sq * 512:(sq + 1) * 512],
                                            start=True, stop=True,
                                        )
                                    pt = ppool.tile([128, 1024], BF16,
                                                    tag=f"p{hh}")
                                    nc.scalar.activation(pt[:, 0:w], sc[:, 0:w],
                                                         Exp, scale=0.125)
                                    if j // 4 == valid[0]:
                                        # diagonal: zero where k > q
                                        nc.gpsimd.affine_select(
                                            out=pt[:, 0:512], in_=pt[:, 0:512],
                                            compare_op=mybir.AluOpType.is_ge,
                                            fill=0.0,
                                            base=valid[0] * 512 - j * 128,
                                            channel_multiplier=-1,
                                            pattern=[[1, 512]],
                                        )
                                    for si, sq in enumerate(valid):
                                        nc.tensor.matmul(
                                            pv_ps[(hh, sq)][:],
                                            v_sb[j][:, hp * 2 + hh, :],
                                            pt[:, si * 512:(si + 1) * 512],
                                            start=(j == 0),
                                            stop=(j == sq * 4 + 3),
                                        )
                            # division: ot = pv[0:64] / pv[64]
                            for hh in range(2):
                                for sq in sqs:
                                    rec = dpool.tile([1, 512], F32, tag="rec")
                                    nc.vector.reciprocal(
                                        rec[:], pv_ps[(hh, sq)][HD:HD + 1, :])
                                    bc = dpool.tile([64, 512], F32, tag="bc")
                                    nc.gpsimd.partition_broadcast(bc[:], rec[:])
                                    nc.vector.tensor_mul(
                                        ot[hp][hh * 64:(hh + 1) * 64,
                                               sq * 512:(sq + 1) * 512],
                                        pv_ps[(hh, sq)][0:HD, :], bc[:])

                # ================= phase 3: output projection =================
                with tc.tile_pool(name="ps_o", bufs=4, space="PSUM") as ps_o, \
                     tc.tile_pool(name="ostage", bufs=3) as ospool:
                    for st in range(NST):
                        stage = ospool.tile([128, D], F32, tag="ostage")
                        for nh in range(2):
                            acc = ps_o.tile([128, 512], F32, tag="oacc")
                            for m in range(2):
                                nc.tensor.matmul(
                                    acc[:],
                                    ot[m][:, st * 128:(st + 1) * 128],
                                    wo_tiles[m][:, nh * 512:(nh + 1) * 512],
                                    start=(m == 0), stop=(m == 1),
                                )
                            nc.vector.tensor_copy(
                                stage[:, nh * 512:(nh + 1) * 512], acc[:])
                        nc.sync.dma_start(
                            out=out_p[st * 128:(st + 1) * 128, :],
                            in_=stage[:])

    nc.compile()
    return nc


def _prep_core_inputs(x, freqs_cos, freqs_sin, Wq, Wk, Wv, Wo, core):
    b, hg = core // 4, core % 4
    hsl = slice(hg * HS, (hg + 1) * HS)
    perm = np.concatenate([np.arange(0, HD, 2), np.arange(1, HD, 2)])

    def permute_heads(w):     # w: [HS, D] -> rope-permuted rows
        return w.reshape(HPC, HD, D)[:, perm, :].reshape(HS, D)

    cosT = freqs_cos.T                      # [32, S]
    sinT = freqs_sin.T
    cosE = np.tile(cosT, (4, 1))            # [128, S]
    sinE = np.concatenate([sinT, -sinT, sinT, -sinT], axis=0)  # sinE' (pre-swap)
    swap = (np.arange(128) // 64) * 64 + ((np.arange(128) % 64 + 32) % 64)
    pswap = np.zeros((128, 128), dtype=np.float32)
    pswap[np.arange(128), swap] = 1.0

    return {
        "xT": np.ascontiguousarray(x[b].T),
        "wq_t": np.ascontiguousarray(permute_heads(Wq[hsl]).T),
        "wk_t": np.ascontiguousarray(permute_heads(Wk[hsl]).T),
        "wv_t": np.ascontiguousarray(Wv[hsl].T),
        "wo_t": np.ascontiguousarray(Wo[:, hsl].T),
        "cosE": np.ascontiguousarray(cosE),
        "sinE": np.ascontiguousarray(sinE),
        "pswap": pswap,
        "tick": np.zeros((1, 1), np.float32),
    }


def kernel(x, freqs_cos, freqs_sin, Wq, Wk, Wv, Wo):
    from concourse.bass_utils import run_bass_kernel_spmd

    x = np.asarray(x, np.float32)
    freqs_cos = np.asarray(freqs_cos, np.float32)
    freqs_sin = np.asarray(freqs_sin, np.float32)
    Wq, Wk, Wv, Wo = (np.asarray(w, np.float32) for w in (Wq, Wk, Wv, Wo))

    if "nc" not in _cached:
        _cached["nc"] = _build_nc()
    nc = _cached["nc"]

    in_maps = [
        _prep_core_inputs(x, freqs_cos, freqs_sin, Wq, Wk, Wv, Wo, c)
        for c in range(N_CORES)
    ]
    res = run_bass_kernel_spmd(nc, in_maps, list(range(N_CORES)))
    out = np.zeros((B, S, D), np.float32)
    for c in range(N_CORES):
        out[c // 4] += res.results[c]["out_p"]
    return out

